# revision 7
# baseline (speedup 1.0000x reference)
"""CTGAN generator forward pass on 8 Trainium2 NeuronCores.

Data-parallel over the batch (65536 rows -> 8192 per core).  All weights are
replicated.  Batch-norm uses exact full-batch statistics via two tiny
cross-core AllReduces of per-core moments.

Device layout is feature-major ("transposed"): activations live in SBUF as
[features(partitions), rows(free)] so that every matmul contraction is over
the partition dim, batch-norm statistics are free-dim reductions, and the
ragged gumbel-softmax segment reductions become small mask matmuls on the PE.

Gumbel softmax per segment with ragged sizes, via a two-pass LSE bound:
    t   = (logits + g + bout) / tau           (tau = 0.2)
    S1  = seg_sum(exp(t/8))                   (overflow-safe: |t|/8 << 88)
    m   = 8*ln(S1)  >= seg_max(t)             (within +8*ln(n) of the max)
    e   = exp(t - m_bcast)                    (in (0, 1])
    out = e / seg_sum(e)_bcast
Segment sums/broadcasts are PE matmuls against constant 0/1 mask matrices.
The 10 alpha columns are overwritten with tanh(logits+bout) via a
per-partition 0/1 mask (features sit on partitions).
"""

import numpy as np

import concourse.bass as bass
import concourse.bacc as bacc
import concourse.tile as tile
import concourse.mybir as mybir
from concourse.bass_utils import run_bass_kernel_spmd

# ---- static problem config (from the CTGAN metadata; fixed for this problem) ----
NUM_CLUSTERS = [10, 8, 10, 5, 10, 10, 7, 10, 10, 10]
NUM_CATEGORIES = [2, 5, 10, 3, 50, 4, 2, 8, 100, 6, 2, 3, 12, 5, 2, 30, 4, 9, 2, 7]
TAU = 0.2
BN_EPS = 1e-3
LATENT = 128
BATCH = 65536
N_CORES = 8
B_LOC = BATCH // N_CORES          # 8192 rows per core
TN = 512                          # rows per device tile (fp32 matmul max free dim)
N_TILES = B_LOC // TN             # 16
D_PAD = 384                       # feature dim padded to 3 partition chunks
N_CHUNK = 3

_seg, _alpha = [], []
_gid = 0
for _c in NUM_CLUSTERS:
    _seg.append(_gid); _alpha.append(True); _gid += 1
    _seg += [_gid] * _c; _alpha += [False] * _c; _gid += 1
for _k in NUM_CATEGORIES:
    _seg += [_gid] * _k; _alpha += [False] * _k; _gid += 1
SEG = np.array(_seg, dtype=np.int32)
ALPHA_MASK = np.array(_alpha, dtype=bool)
N_SEG = _gid                      # 40
DATA_DIM = len(_seg)              # 366
assert DATA_DIM == 366 and N_SEG == 40

F32 = mybir.dt.float32

_COMPILED = None  # cached (nc, names) so repeated kernel() calls reuse the program


def _build_program():
    nc = bacc.Bacc("TRN2", target_bir_lowering=False, debug=False, num_devices=N_CORES)

    # ---- per-core DRAM I/O (feature-major) ----
    zT = nc.dram_tensor("zT", [LATENT, B_LOC], F32, kind="ExternalInput")
    gT = nc.dram_tensor("gT", [D_PAD, B_LOC], F32, kind="ExternalInput")       # (g + bout).T, padded
    w1 = nc.dram_tensor("w1", [LATENT, 256], F32, kind="ExternalInput")
    w2 = nc.dram_tensor("w2", [384, 256], F32, kind="ExternalInput")
    wout = nc.dram_tensor("wout", [640, D_PAD], F32, kind="ExternalInput")     # padded cols
    gb1 = nc.dram_tensor("gb1", [2, 256], F32, kind="ExternalInput")           # gamma1;beta1
    gb2 = nc.dram_tensor("gb2", [2, 256], F32, kind="ExternalInput")
    mseg = nc.dram_tensor("mseg", [D_PAD, N_SEG], F32, kind="ExternalInput")   # 0/1
    msegt = nc.dram_tensor("msegt", [2 * N_SEG, D_PAD], F32, kind="ExternalInput")  # rows 0-39: -1.6*mask, 40-79: mask
    amask = nc.dram_tensor("amask", [128, 1], F32, kind="ExternalInput")       # alpha cols (all < 128)
    boutc = nc.dram_tensor("boutc", [128, 1], F32, kind="ExternalInput")       # bout chunk0
    outT = nc.dram_tensor("outT", [DATA_DIM, B_LOC], F32, kind="ExternalOutput")

    with tile.TileContext(nc) as tc:
        with (
            tc.tile_pool(name="singles", bufs=1) as singles,
            tc.tile_pool(name="dram", bufs=1, space="DRAM") as drampool,
        ):
            # ---- persistent weights/constants in SBUF ----
            w1_s = singles.tile([128, 256], F32)
            nc.sync.dma_start(out=w1_s, in_=w1[:, :])
            w2_s = singles.tile([128, 3, 256], F32)
            nc.sync.dma_start(out=w2_s, in_=w2.ap().rearrange("(c p) m -> p c m", p=128))
            wo_s = singles.tile([128, 5, D_PAD], F32)
            nc.sync.dma_start(out=wo_s, in_=wout.ap().rearrange("(c p) m -> p c m", p=128))
            mseg_s = singles.tile([128, N_CHUNK, N_SEG], F32)
            nc.sync.dma_start(out=mseg_s, in_=mseg.ap().rearrange("(c p) m -> p c m", p=128))
            msegt_s = singles.tile([N_SEG, 2, D_PAD], F32)
            nc.sync.dma_start(out=msegt_s, in_=msegt.ap().rearrange("(b s) m -> s b m", s=N_SEG))
            amask_s = singles.tile([128, 1], F32)
            nc.sync.dma_start(out=amask_s, in_=amask[:, :])
            bout_s = singles.tile([128, 1], F32)
            nc.sync.dma_start(out=bout_s, in_=boutc[:, :])
            gb1_s = singles.tile([128, 2, 2], F32)   # [p, {gamma,beta}, chunk]
            nc.sync.dma_start(out=gb1_s, in_=gb1.ap().rearrange("b (c p) -> p b c", p=128))
            gb2_s = singles.tile([128, 2, 2], F32)
            nc.sync.dma_start(out=gb2_s, in_=gb2.ap().rearrange("b (c p) -> p b c", p=128))
            eps_s = singles.tile([128, 1], F32)
            nc.vector.memset(eps_s, BN_EPS)

            # persistent activations
            h1c_s = singles.tile([128, 2, B_LOC], F32)   # relu(bn1(h1)), feature-major
            h2_s = singles.tile([128, 2, B_LOC], F32)    # raw h2

            # BN affine params (filled after each AllReduce)
            sc1 = singles.tile([128, 2], F32)
            sh1 = singles.tile([128, 2], F32)
            sc2 = singles.tile([128, 2], F32)
            sh2 = singles.tile([128, 2], F32)

            stats1 = singles.tile([128, 2, N_TILES, 6], F32)
            stats2 = singles.tile([128, 2, N_TILES, 6], F32)

            def finalize_stats(stats_buf, gb_s, sc_t, sh_t, tag):
                """bn_aggr per chunk -> convert to sums -> AllReduce -> scale/shift."""
                ex = singles.tile([128, 4], F32, tag=f"ex{tag}")
                mv = singles.tile([128, 2, 2], F32, tag=f"mv{tag}")
                tmp = singles.tile([128, 1], F32, tag=f"tmp{tag}")
                for c in range(2):
                    nc.vector.bn_aggr(out=mv[:, c, :], in_=stats_buf[:, c, :, :])
                    # s = mean * n_loc ; q = (var + mean^2) * n_loc
                    nc.vector.tensor_scalar_mul(out=ex[:, 2 * c:2 * c + 1], in0=mv[:, c, 0:1], scalar1=float(B_LOC))
                    nc.vector.tensor_mul(out=tmp, in0=mv[:, c, 0:1], in1=mv[:, c, 0:1])
                    nc.vector.tensor_add(out=tmp, in0=tmp, in1=mv[:, c, 1:2])
                    nc.vector.tensor_scalar_mul(out=ex[:, 2 * c + 1:2 * c + 2], in0=tmp, scalar1=float(B_LOC))
                in_b = drampool.tile([128, 4], F32, tag=f"arin{tag}")
                out_b = drampool.tile([128, 4], F32, tag=f"arout{tag}")
                nc.gpsimd.dma_start(in_b[:], ex)
                nc.gpsimd.collective_compute(
                    "AllReduce", mybir.AluOpType.add,
                    replica_groups=[list(range(N_CORES))],
                    ins=[in_b.opt()], outs=[out_b.opt()],
                )
                gx = singles.tile([128, 4], F32, tag=f"gx{tag}")
                nc.gpsimd.dma_start(gx, out_b[:])
                mu = singles.tile([128, 2], F32, tag=f"mu{tag}")
                var = singles.tile([128, 1], F32, tag=f"var{tag}")
                for c in range(2):
                    nc.vector.tensor_scalar_mul(out=mu[:, c:c + 1], in0=gx[:, 2 * c:2 * c + 1], scalar1=1.0 / BATCH)
                    nc.vector.tensor_scalar_mul(out=var, in0=gx[:, 2 * c + 1:2 * c + 2], scalar1=1.0 / BATCH)
                    nc.vector.tensor_mul(out=tmp, in0=mu[:, c:c + 1], in1=mu[:, c:c + 1])
                    nc.vector.tensor_sub(out=var, in0=var, in1=tmp)
                    # std = sqrt(var + eps); rsig = 1/std
                    nc.scalar.activation(out=var, in_=var, func=mybir.ActivationFunctionType.Sqrt, bias=eps_s, scale=1.0)
                    nc.vector.reciprocal(out=var, in_=var)
                    nc.vector.tensor_mul(out=sc_t[:, c:c + 1], in0=gb_s[:, 0, c:c + 1], in1=var)
                    nc.vector.tensor_mul(out=tmp, in0=mu[:, c:c + 1], in1=sc_t[:, c:c + 1])
                    nc.vector.tensor_sub(out=sh_t[:, c:c + 1], in0=gb_s[:, 1, c:c + 1], in1=tmp)

            # ================= phase 1: stats of h1 = zT.T@W1 =================
            with (
                tc.tile_pool(name="p1z", bufs=2) as p1z,
                tc.tile_pool(name="p1ps", bufs=2, space="PSUM") as p1ps,
            ):
                for i in range(N_TILES):
                    zt = p1z.tile([128, TN], F32, tag="zt")
                    nc.sync.dma_start(out=zt, in_=zT[:, i * TN:(i + 1) * TN])
                    h1 = p1ps.tile([128, 2, TN], F32, tag="h1")
                    for m in range(2):
                        nc.tensor.matmul(h1[:, m, :], w1_s[:, m * 128:(m + 1) * 128], zt, start=True, stop=True)
                    for c in range(2):
                        nc.vector.bn_stats(out=stats1[:, c, i, :], in_=h1[:, c, :])
            finalize_stats(stats1, gb1_s, sc1, sh1, "1")

            # ====== phase 2: h1c = relu(bn1), h2 = h1c.T@W2 (stats only) ======
            with (
                tc.tile_pool(name="p2z", bufs=2) as p2z,
                tc.tile_pool(name="p2ps", bufs=2, space="PSUM") as p2ps,
            ):
                for i in range(N_TILES):
                    ts_ = slice(i * TN, (i + 1) * TN)
                    zt = p2z.tile([128, TN], F32, tag="zt")
                    nc.sync.dma_start(out=zt, in_=zT[:, ts_])
                    h1 = p2ps.tile([128, 2, TN], F32, tag="h1")
                    for m in range(2):
                        nc.tensor.matmul(h1[:, m, :], w1_s[:, m * 128:(m + 1) * 128], zt, start=True, stop=True)
                    for c in range(2):
                        nc.scalar.activation(
                            out=h1c_s[:, c, ts_], in_=h1[:, c, :],
                            func=mybir.ActivationFunctionType.Relu,
                            bias=sh1[:, c:c + 1], scale=sc1[:, c:c + 1],
                        )
                    h2 = p2ps.tile([128, 2, TN], F32, tag="h2")
                    for m in range(2):
                        for k in range(3):
                            rhs = zt if k == 2 else h1c_s[:, k, ts_]
                            nc.tensor.matmul(h2[:, m, :], w2_s[:, k, m * 128:(m + 1) * 128], rhs,
                                             start=(k == 0), stop=(k == 2))
                    for c in range(2):
                        nc.vector.bn_stats(out=stats2[:, c, i, :], in_=h2[:, c, :])
                        nc.vector.tensor_copy(out=h2_s[:, c, ts_], in_=h2[:, c, :])
            finalize_stats(stats2, gb2_s, sc2, sh2, "2")

            # ================= phase 3: logits + gumbel softmax =================
            with (
                tc.tile_pool(name="p3z", bufs=2) as p3z,
                tc.tile_pool(name="p3g", bufs=3) as p3g,
                tc.tile_pool(name="p3w", bufs=2) as p3w,
                tc.tile_pool(name="p3s", bufs=2) as p3s,
                tc.tile_pool(name="p3lg", bufs=4, space="PSUM") as p3lg,
                tc.tile_pool(name="p3bc", bufs=2, space="PSUM") as p3bc,
                tc.tile_pool(name="p3sg", bufs=1, space="PSUM") as p3sg,
            ):
                for i in range(N_TILES):
                    ts_ = slice(i * TN, (i + 1) * TN)
                    zt = p3z.tile([128, TN], F32, tag="zt")
                    nc.sync.dma_start(out=zt, in_=zT[:, ts_])
                    # h2c chunks = relu(bn2(h2))
                    h2c = p3w.tile([128, 2, TN], F32, tag="h2c")
                    for c in range(2):
                        nc.scalar.activation(
                            out=h2c[:, c, :], in_=h2_s[:, c, ts_],
                            func=mybir.ActivationFunctionType.Relu,
                            bias=sh2[:, c:c + 1], scale=sc2[:, c:c + 1],
                        )
                    rhs_list = [h2c[:, 0, :], h2c[:, 1, :], h1c_s[:, 0, ts_], h1c_s[:, 1, ts_], zt]

                    s0s, eas, lgps = [], [], []
                    vtile = p3w.tile([128, TN], F32, tag="v")
                    S1 = p3sg.tile([N_SEG, TN], F32, tag="S1")
                    for c in range(N_CHUNK):
                        lg = p3lg.tile([128, TN], F32, tag="lg")
                        for k in range(5):
                            nc.tensor.matmul(lg, wo_s[:, k, c * 128:(c + 1) * 128], rhs_list[k],
                                             start=(k == 0), stop=(k == 4))
                        gt = p3g.tile([128, TN], F32, tag="gt")
                        nc.sync.dma_start(out=gt, in_=gT[c * 128:(c + 1) * 128, ts_])
                        s0 = p3s.tile([128, TN], F32, tag=f"s0{c}")
                        nc.vector.tensor_add(out=s0, in0=lg, in1=gt)          # logits + g + bout
                        if c == 0:
                            nc.vector.tensor_scalar_add(out=vtile, in0=lg, scalar1=bout_s)  # logits + bout
                        ea = p3s.tile([128, TN], F32, tag=f"ea{c}")
                        nc.scalar.activation(out=ea, in_=s0, func=mybir.ActivationFunctionType.Exp,
                                             scale=1.0 / (8.0 * TAU))         # exp(t/8), t = s0/tau
                        nc.tensor.matmul(S1, mseg_s[:, c, :], ea, start=(c == 0), stop=(c == 2))
                        s0s.append(s0); eas.append(ea); lgps.append(lg)

                    lnS1 = p3s.tile([N_SEG, TN], F32, tag="lnS1")
                    nc.scalar.activation(out=lnS1, in_=S1, func=mybir.ActivationFunctionType.Ln)

                    S2 = p3sg.tile([N_SEG, TN], F32, tag="S2")
                    for c in range(N_CHUNK):
                        bc = p3bc.tile([128, TN], F32, tag="bc")
                        nc.tensor.matmul(bc, msegt_s[:, 0, c * 128:(c + 1) * 128], lnS1, start=True, stop=True)
                        # u = s0 + (-8/5)*lnS1[seg] ; e = exp(u/tau ... ) -> exp(t - 8 lnS1)
                        nc.vector.tensor_add(out=s0s[c], in0=s0s[c], in1=bc)
                        nc.scalar.activation(out=eas[c], in_=s0s[c], func=mybir.ActivationFunctionType.Exp,
                                             scale=1.0 / TAU)
                        nc.tensor.matmul(S2, mseg_s[:, c, :], eas[c], start=(c == 0), stop=(c == 2))

                    r2 = p3s.tile([N_SEG, TN], F32, tag="r2")
                    nc.vector.reciprocal(out=r2, in_=S2)
                    for c in range(N_CHUNK):
                        bc = p3bc.tile([128, TN], F32, tag="bc")
                        nc.tensor.matmul(bc, msegt_s[:, 1, c * 128:(c + 1) * 128], r2, start=True, stop=True)
                        nc.vector.tensor_mul(out=s0s[c], in0=eas[c], in1=bc)  # softmax result
                    # alpha columns: overwrite with tanh(logits + bout) (chunk 0 only)
                    nc.scalar.activation(out=vtile, in_=vtile, func=mybir.ActivationFunctionType.Tanh)
                    nc.vector.tensor_sub(out=vtile, in0=vtile, in1=s0s[0])
                    nc.vector.scalar_tensor_tensor(out=s0s[0], in0=vtile, scalar=amask_s, in1=s0s[0],
                                                   op0=mybir.AluOpType.mult, op1=mybir.AluOpType.add)
                    for c in range(N_CHUNK):
                        rows = 128 if c < 2 else DATA_DIM - 256
                        nc.sync.dma_start(out=outT[c * 128:c * 128 + rows, ts_], in_=s0s[c][0:rows, :])

    nc.compile()
    return nc


def _prepare_core_inputs(z, g, W1, b1, gamma1, beta1, W2, b2, gamma2, beta2, Wout, bout):
    z = np.asarray(z, np.float32); g = np.asarray(g, np.float32)
    Wout = np.asarray(Wout, np.float32); bout = np.asarray(bout, np.float32)

    zT = np.ascontiguousarray(z.T)                          # [128, BATCH]
    g_eff = g + bout[None, :].astype(np.float32)
    gT = np.zeros((D_PAD, BATCH), np.float32)
    gT[:DATA_DIM] = g_eff.T

    wout_p = np.zeros((640, D_PAD), np.float32)
    wout_p[:, :DATA_DIM] = Wout

    mseg = np.zeros((D_PAD, N_SEG), np.float32)
    mseg[np.arange(DATA_DIM), SEG] = 1.0
    msegt = np.zeros((2 * N_SEG, D_PAD), np.float32)
    msegt[SEG, np.arange(DATA_DIM)] = -8.0 * TAU            # -1.6 => bcast of -(8/5)*lnS1
    msegt[N_SEG + SEG, np.arange(DATA_DIM)] = 1.0
    amask = np.zeros((128, 1), np.float32)
    apos = np.nonzero(ALPHA_MASK)[0]
    assert apos.max() < 128
    amask[apos, 0] = 1.0
    boutc = np.ascontiguousarray(bout[:128].astype(np.float32).reshape(128, 1))

    gb1 = np.stack([np.asarray(gamma1, np.float32), np.asarray(beta1, np.float32)])
    gb2 = np.stack([np.asarray(gamma2, np.float32), np.asarray(beta2, np.float32)])

    shared = {
        "w1": np.ascontiguousarray(np.asarray(W1, np.float32)),
        "w2": np.ascontiguousarray(np.asarray(W2, np.float32)),
        "wout": wout_p,
        "gb1": np.ascontiguousarray(gb1), "gb2": np.ascontiguousarray(gb2),
        "mseg": mseg, "msegt": msegt, "amask": amask, "boutc": boutc,
    }
    in_maps = []
    for c in range(N_CORES):
        sl = slice(c * B_LOC, (c + 1) * B_LOC)
        m = dict(shared)
        m["zT"] = np.ascontiguousarray(zT[:, sl])
        m["gT"] = np.ascontiguousarray(gT[:, sl])
        in_maps.append(m)
    return in_maps


def get_program():
    global _COMPILED
    if _COMPILED is None:
        _COMPILED = _build_program()
    return _COMPILED


_RUNNER = None


def get_runner():
    """Build (once) a cached jitted SPMD callable over the 8 cores.

    Returns (fn, in_names, out_names, out_avals).  fn takes the per-core
    inputs concatenated along axis 0 (global arrays), plus zero-filled
    donated output buffers, and returns the concatenated outputs.
    """
    global _RUNNER
    if _RUNNER is not None:
        return _RUNNER
    import jax
    from jax.sharding import Mesh, PartitionSpec
    from jax.experimental.shard_map import shard_map
    import concourse.mybir as mybir_
    from concourse import bass2jax

    nc = get_program()
    bass2jax.install_neuronx_cc_hook()
    partition_name = nc.partition_id_tensor.name if nc.partition_id_tensor else None
    in_names, out_names, out_avals = [], [], []
    for alloc in nc.m.functions[0].allocations:
        if not isinstance(alloc, mybir_.MemoryLocationSet):
            continue
        name = alloc.memorylocations[0].name
        if alloc.kind == "ExternalInput":
            if name != partition_name:
                in_names.append(name)
        elif alloc.kind == "ExternalOutput":
            out_names.append(name)
            out_avals.append(jax.core.ShapedArray(tuple(alloc.tensor_shape), mybir_.dt.np(alloc.dtype)))
    n_params = len(in_names)
    all_in_names = list(in_names) + list(out_names)
    if partition_name is not None:
        all_in_names.append(partition_name)
    donate = tuple(range(n_params, n_params + len(out_names)))

    def _body(*args):
        operands = list(args)
        if partition_name is not None:
            operands.append(bass2jax.partition_id_tensor())
        outs = bass2jax._bass_exec_p.bind(
            *operands,
            out_avals=tuple(out_avals),
            in_names=tuple(all_in_names),
            out_names=tuple(out_names),
            lowering_input_output_aliases=(),
            sim_require_finite=True,
            sim_require_nnan=True,
            nc=nc,
        )
        return tuple(outs)

    devices = jax.devices()[:N_CORES]
    mesh = Mesh(np.asarray(devices), ("core",))
    in_specs = (PartitionSpec("core"),) * (n_params + len(out_names))
    out_specs = (PartitionSpec("core"),) * len(out_names)
    fn = jax.jit(
        shard_map(_body, mesh=mesh, in_specs=in_specs, out_specs=out_specs, check_rep=False),
        donate_argnums=donate, keep_unused=True,
    )
    _RUNNER = (fn, in_names, out_names, out_avals)
    return _RUNNER


def concat_inputs(in_maps):
    fn, in_names, out_names, out_avals = get_runner()
    return [np.concatenate([np.asarray(m[name]) for m in in_maps], axis=0) for name in in_names]


def make_zero_outs():
    fn, in_names, out_names, out_avals = get_runner()
    return [np.zeros((N_CORES * a.shape[0], *a.shape[1:]), a.dtype) for a in out_avals]


def run(in_maps):
    """Execute on the 8 cores; returns {name: [per-core arrays]}."""
    fn, in_names, out_names, out_avals = get_runner()
    out_arrs = fn(*concat_inputs(in_maps), *make_zero_outs())
    res = {}
    for i, name in enumerate(out_names):
        glob = np.asarray(out_arrs[i]).reshape(N_CORES, *out_avals[i].shape)
        res[name] = [glob[c] for c in range(N_CORES)]
    return res


def kernel(**inputs) -> np.ndarray:
    in_maps = _prepare_core_inputs(**inputs)
    res = run(in_maps)
    out = np.empty((BATCH, DATA_DIM), np.float32)
    for c in range(N_CORES):
        out[c * B_LOC:(c + 1) * B_LOC, :] = res["outT"][c].T
    return out
